# revision 1
# baseline (speedup 1.0000x reference)
"""Causal self-attention (RoPE) Trainium2 Bass kernel.

Sharding: 8 cores = 4 batches x 2 head-groups. Core c handles batch c//2 and
heads (c%2)*8 .. (c%2)*8+7. Each core computes its QKV projection slice, RoPE,
causal flash-style attention in transposed layout, and a partial output
projection; the host sums the two partial projections per batch.

All matmuls use float32r (TF32-like, ~1.5e-4 rel err) at full PE rate.
Attention is computed transposed (s^T = k q^T) so softmax denominators come
from an appended ones-column in the value matrix and attention output feeds
the output projection as lhsT with no transposes.
"""

import math
import numpy as np
from contextlib import ExitStack

import concourse.bass as bass
import concourse.tile as tile
from concourse import bacc, mybir
from concourse.bass_utils import run_bass_kernel_spmd

F32 = mybir.dt.float32
R32 = mybir.dt.float32r
EXPF = mybir.ActivationFunctionType.Exp
MULT = mybir.AluOpType.mult
ADD = mybir.AluOpType.add

B, T, C, H, D = 4, 2048, 1024, 16, 64
HL = 8            # local heads per core
NP = HL // 2      # head pairs per core
KT = C // 128     # contraction tiles for projections
TT = T // 128     # 128-row tiles of T
QC = T // 512     # 512-col chunks of T
SCALE = 1.0 / math.sqrt(D)

_CACHE = {}


def _build_nc():
    nc = bacc.Bacc("TRN2", debug=False, num_devices=8)

    xT_d = nc.dram_tensor("xT", [KT, 128, T], R32, kind="ExternalInput").ap()
    wq_d = nc.dram_tensor("wq", [128, NP, KT, 128], R32, kind="ExternalInput").ap()
    wk_d = nc.dram_tensor("wk", [128, NP, KT, 128], R32, kind="ExternalInput").ap()
    wv_d = nc.dram_tensor("wv", [128, KT, 512], R32, kind="ExternalInput").ap()
    wo_d = nc.dram_tensor("wo", [128, NP, C], R32, kind="ExternalInput").ap()
    cos_d = nc.dram_tensor("cosT", [128, T], F32, kind="ExternalInput").ap()
    sin_d = nc.dram_tensor("sinT", [128, T], F32, kind="ExternalInput").ap()
    psw_d = nc.dram_tensor("psw", [128, 128], R32, kind="ExternalInput").ap()
    e64_d = nc.dram_tensor("e64", [128, 64], R32, kind="ExternalInput").ap()
    msk_d = nc.dram_tensor("msk", [128, 4, 512], F32, kind="ExternalInput").ap()
    wrm_d = nc.dram_tensor("wrm", [128, 512], R32, kind="ExternalInput").ap()
    out_d = nc.dram_tensor("out", [T, C], F32, kind="ExternalOutput").ap()

    with tile.TileContext(nc) as tc:
        with ExitStack() as ctx:
            pers = ctx.enter_context(tc.tile_pool(name="pers", bufs=1))
            vext = pers.tile([128, TT, HL, D + 1], R32)
            qkT = {}
            for p in range(NP):
                for s in "qk":
                    qkT[(p, s)] = pers.tile([128, T], R32, name=f"qkT_{p}_{s}")
            yT = [pers.tile([128, T], R32, name=f"yT_{r}") for r in range(NP)]
            nc.gpsimd.memset(vext[:, :, :, D].bitcast(F32), 1.0)

            # ---- phase V: value projection -> vext (natural layout + ones col)
            with (
                tc.tile_pool(name="vph", bufs=2) as vp,
                tc.tile_pool(name="vw", bufs=1) as vw,
                tc.tile_pool(name="vps", bufs=2, space="PSUM") as vps,
            ):
                wv_sb = vw.tile([128, KT, 512], R32)
                nc.sync.dma_start(wv_sb[:], wv_d)
                for tt in range(TT):
                    xv = vp.tile([128, KT, 128], R32, tag="xv")
                    nc.sync.dma_start(
                        xv[:],
                        xT_d[:, :, tt * 128 : (tt + 1) * 128].rearrange(
                            "k c t -> c k t"
                        ),
                    )
                    ps = vps.tile([128, 512], F32, tag="pv")
                    for kt in range(KT):
                        nc.tensor.matmul(
                            ps[:], xv[:, kt], wv_sb[:, kt],
                            start=(kt == 0), stop=(kt == KT - 1),
                        )
                    nc.vector.tensor_copy(vext[:, tt, :, 0:D], ps[:])

            # ---- phase QK: q/k projection + RoPE -> qkT (transposed layout)
            with (
                tc.tile_pool(name="qkst", bufs=2) as sp,
                tc.tile_pool(name="qkw", bufs=1) as qw,
                tc.tile_pool(name="ctab", bufs=2) as ct,
                tc.tile_pool(name="qkps", bufs=2, space="PSUM") as qps,
                tc.tile_pool(name="rotps", bufs=2, space="PSUM") as rps,
            ):
                wq_sb = qw.tile([128, NP, KT, 128], R32)
                wk_sb = qw.tile([128, NP, KT, 128], R32)
                psw_sb = qw.tile([128, 128], R32)
                nc.sync.dma_start(wq_sb[:], wq_d)
                nc.sync.dma_start(wk_sb[:], wk_d)
                nc.sync.dma_start(psw_sb[:], psw_d)
                for qc in range(QC):
                    lo, hi = qc * 512, (qc + 1) * 512
                    xc = sp.tile([128, KT, 512], R32, tag="xc")
                    nc.sync.dma_start(
                        xc[:], xT_d[:, :, lo:hi].rearrange("k c t -> c k t")
                    )
                    cosc = ct.tile([128, 512], F32, tag="cosc")
                    sinc = ct.tile([128, 512], F32, tag="sinc")
                    nc.sync.dma_start(cosc[:], cos_d[:, lo:hi])
                    nc.sync.dma_start(sinc[:], sin_d[:, lo:hi])
                    for p in range(NP):
                        for w_sb, key in ((wq_sb, "q"), (wk_sb, "k")):
                            dst = qkT[(p, key)][:, lo:hi]
                            ps = qps.tile([128, 512], F32, tag="pq")
                            for kt in range(KT):
                                nc.tensor.matmul(
                                    ps[:], w_sb[:, p, kt], xc[:, kt],
                                    start=(kt == 0), stop=(kt == KT - 1),
                                )
                            nc.vector.tensor_tensor(
                                dst, ps[:], cosc[:], MULT
                            )
                            u = sp.tile([128, 512], R32, tag="u")
                            nc.vector.tensor_tensor(
                                u[:], ps[:], sinc[:], MULT
                            )
                            pr = rps.tile([128, 512], F32, tag="pr")
                            nc.tensor.matmul(
                                pr[:], psw_sb[:], u[:], start=True, stop=True
                            )
                            nc.vector.tensor_tensor(dst, pr[:], dst, ADD)

            # ---- phase ATT: causal attention per head pair, transposed
            with (
                tc.tile_pool(name="attp", bufs=3) as ap_,
                tc.tile_pool(name="atab", bufs=1) as at_,
                tc.tile_pool(name="sps", bufs=2, space="PSUM") as sps,
                tc.tile_pool(name="yps", bufs=2, space="PSUM") as yps,
            ):
                e64_sb = at_.tile([128, 64], R32)
                msk_sb = at_.tile([128, 4, 512], F32)
                rrec = at_.tile([128, 512], R32)
                rscr = at_.tile([128, 512], F32)
                rscr2 = at_.tile([128, 512], F32)
                ww_sb = at_.tile([128, 128], R32)
                wrm_sb = at_.tile([128, 512], R32)
                nc.sync.dma_start(e64_sb[:], e64_d)
                nc.sync.dma_start(msk_sb[:], msk_d)
                nc.sync.dma_start(ww_sb[:], psw_d)
                nc.sync.dma_start(wrm_sb[:], wrm_d)
                nc.gpsimd.memset(rrec[:].bitcast(F32), 0.0)
                for p in range(NP):
                    qTt = qkT[(p, "q")]
                    kTt = qkT[(p, "k")]
                    for qc in range(QC):
                        lo, hi = qc * 512, (qc + 1) * 512
                        nkt = (qc + 1) * 4
                        psyA = yps.tile([65, 512], F32, tag="yA")
                        psyB = yps.tile([65, 512], F32, tag="yB")
                        for kt in range(nkt):
                            first, last = kt == 0, kt == nkt - 1
                            klo, khi = kt * 128, (kt + 1) * 128
                            off = klo - lo
                            # valid q range for this tile is [off, 512); trim
                            # to it when the fp32r fast path allows (>=256)
                            tr = off if off in (128, 256) else 0
                            w = 512 - tr
                            ps2 = sps.tile([128, 1024], F32, tag="sA")
                            p3 = ps2[:].rearrange("p (h n) -> p h n", h=2)
                            nc.tensor.matmul(
                                ps2[:, tr:512],
                                kTt[0:64, klo:khi], qTt[0:64, lo + tr : hi],
                                start=True, stop=True,
                            )
                            nc.tensor.matmul(
                                ps2[:, 512 + tr : 1024],
                                kTt[64:128, klo:khi], qTt[64:128, lo + tr : hi],
                                start=True, stop=True,
                            )
                            aAB = ap_.tile([128, 1024], R32, tag="aA")
                            a3 = aAB[:].rearrange("p (h n) -> p h n", h=2)
                            aA = aAB[:, 0:512]
                            aB = aAB[:, 512:1024]
                            nc.scalar.activation(
                                a3[:, :, tr:512], p3[:, :, tr:512],
                                EXPF, scale=SCALE,
                            )
                            if off >= 0:
                                mi = off // 128
                                nc.vector.tensor_tensor(
                                    aA[:, tr:512], aA[:, tr:512],
                                    msk_sb[:, mi, tr:512], MULT,
                                )
                                nc.vector.tensor_tensor(
                                    aB[:, tr:512], aB[:, tr:512],
                                    msk_sb[:, mi, tr:512], MULT,
                                )
                            nc.tensor.matmul(
                                psyA[:, tr:512], vext[:, kt, 2 * p, :],
                                aA[:, tr:512], start=first, stop=last,
                            )
                            nc.tensor.matmul(
                                psyB[:, tr:512], vext[:, kt, 2 * p + 1, :],
                                aB[:, tr:512], start=first, stop=last,
                            )
                        # normalize: recip of denom row, broadcast via selector
                        # matmul, multiply into yT
                        for hh, psy in ((0, psyA), (1, psyB)):
                            with nc.allow_low_precision(
                                reason="recip row feeds fp32r selector matmul"
                            ):
                                nc.vector.reciprocal(
                                    rrec[64:65, :], psy[64:65, :]
                                )
                            pbc = sps.tile([64, 512], F32, tag="sA", name="pbc")
                            nc.tensor.matmul(
                                pbc[:], e64_sb[:], rrec[:], start=True, stop=True
                            )
                            bc = ap_.tile([64, 512], R32, tag="bc")
                            nc.scalar.copy(bc[:], pbc[:])
                            if hh == 0:
                                nc.vector.tensor_tensor(
                                    yT[p][0:64, lo:hi], psy[0:64, :], bc[:], MULT
                                )
                            else:
                                tb = ap_.tile([64, 512], R32, tag="tb")
                                nc.vector.tensor_tensor(
                                    tb[:], psy[0:64, :], bc[:], MULT
                                )
                                nc.sync.dma_start(yT[p][64:128, lo:hi], tb[:])

            # ---- phase OUT: output projection (partial; host sums over cores)
            with (
                tc.tile_pool(name="oph", bufs=3) as op_,
                tc.tile_pool(name="ow", bufs=1) as ow,
                tc.tile_pool(name="ops", bufs=4, space="PSUM") as ops,
            ):
                wo_sb = ow.tile([128, NP, C], R32)
                nc.sync.dma_start(wo_sb[:], wo_d)
                for mt in range(TT):
                    mlo, mhi = mt * 128, (mt + 1) * 128
                    for cc in range(2):
                        clo, chi = cc * 512, (cc + 1) * 512
                        ps = ops.tile([128, 512], F32, tag="po")
                        for r in range(NP):
                            nc.tensor.matmul(
                                ps[:], yT[r][:, mlo:mhi], wo_sb[:, r, clo:chi],
                                start=(r == 0), stop=(r == NP - 1),
                            )
                        ob = op_.tile([128, 512], F32, tag="ob")
                        nc.vector.tensor_copy(ob[:], ps[:])
                        nc.sync.dma_start(out_d[mlo:mhi, clo:chi], ob[:])

    nc.compile()
    return nc


def _host_tables():
    half = D // 2
    freq = np.exp(-math.log(10000.0) * np.arange(half) / half).astype(np.float64)
    ang = np.arange(T, dtype=np.float64)[None, :] * freq[:, None]  # [32, T]
    cos32 = np.cos(ang).astype(np.float32)
    sin32 = np.sin(ang).astype(np.float32)
    cosT = np.tile(cos32, (4, 1))                                   # [128, T]
    sinT = np.concatenate([sin32, -sin32, sin32, -sin32], axis=0)   # [128, T]
    psw = np.zeros((128, 128), np.float32)
    psw[np.arange(128) ^ 32, np.arange(128)] = 1.0
    e64 = np.zeros((128, 64), np.float32)
    e64[64, :] = 1.0
    kk = np.arange(128)[:, None, None]
    ii = np.arange(4)[None, :, None]
    qq = np.arange(512)[None, None, :]
    msk = (qq >= kk + ii * 128).astype(np.float32)
    return cosT, sinT, psw, e64, msk


def _pack_weights(w_qkv, w_out, hg):
    lo, hi = hg * HL, (hg + 1) * HL
    wqf = w_qkv[:, 0:C].reshape(C, H, D)[:, lo:hi]       # [C, 8, D]
    wkf = w_qkv[:, C : 2 * C].reshape(C, H, D)[:, lo:hi]
    wvf = w_qkv[:, 2 * C : 3 * C].reshape(C, H, D)[:, lo:hi]

    def pack_qk(w):
        a = w.reshape(KT, 128, NP, 2, D)
        return np.ascontiguousarray(
            a.transpose(1, 2, 0, 3, 4).reshape(128, NP, KT, 128)
        )

    wq = pack_qk(wqf)
    wk = pack_qk(wkf)
    wv = np.ascontiguousarray(
        wvf.reshape(KT, 128, HL * D).transpose(1, 0, 2)
    )
    wo_l = w_out.reshape(H, D, C)[lo:hi].reshape(NP, 128, C)
    wo = np.ascontiguousarray(wo_l.transpose(1, 0, 2))
    return wq, wk, wv, wo


def kernel(x, w_qkv, w_out):
    x = np.asarray(x, dtype=np.float32)
    w_qkv = np.asarray(w_qkv, dtype=np.float32)
    w_out = np.asarray(w_out, dtype=np.float32)

    if "nc" not in _CACHE:
        _CACHE["nc"] = _build_nc()
    nc = _CACHE["nc"]

    cosT, sinT, psw, e64, msk = _host_tables()
    packs = [_pack_weights(w_qkv, w_out, hg) for hg in range(2)]
    xTs = [
        np.ascontiguousarray(x[b].T).reshape(KT, 128, T) for b in range(B)
    ]

    in_maps = []
    for c in range(8):
        b, hg = c // 2, c % 2
        wq, wk, wv, wo = packs[hg]
        in_maps.append(
            {
                "xT": xTs[b], "wq": wq, "wk": wk, "wv": wv, "wo": wo,
                "cosT": cosT, "sinT": sinT, "psw": psw, "e64": e64,
                "msk": msk, "wrm": np.full((128, 512), 0.03, np.float32),
            }
        )

    res = run_bass_kernel_spmd(nc, in_maps, core_ids=list(range(8)))
    outs = [res.results[c]["out"] for c in range(8)]
    y = np.stack([outs[2 * b] + outs[2 * b + 1] for b in range(B)], axis=0)
    return y.astype(np.float32)



# revision 16
# speedup vs baseline: 1.6775x; 1.6775x over previous
"""Causal self-attention (RoPE) Trainium2 Bass kernel.

Sharding: 8 cores = 4 batches x 2 head-groups. Core c handles batch c//2 and
heads (c%2)*8 .. (c%2)*8+7. Each core computes its QKV projection slice, RoPE,
causal flash-style attention in transposed layout, and a partial output
projection; the host sums the two partial projections per batch.

Projections run in float32r (TF32-like, full PE rate at 512-wide streams);
attention runs in bf16 which lifts the fp32r >=256-wide restriction so causal
tiles trim to their exact valid width. The whole kernel is one software-
pipelined loop over 512-column chunks: projections for chunk qc+1 are emitted
interleaved into the attention of chunk qc so the PE array never drains (and
stays at its top p-state) while the Activation engine works through the
softmax exponentials. Softmax reciprocals use the fast approximate DVE op and
denominator broadcasts ride a tiny selector matmul.
"""

import math
import numpy as np
from contextlib import ExitStack

import ml_dtypes
import concourse.bass as bass
import concourse.tile as tile
from concourse import bacc, mybir
from concourse.bass_utils import run_bass_kernel_spmd
from concourse.dve_ops import RECIP_APPROX_FAST_CONSTS, RECIPROCAL_APPROX_FAST

F32 = mybir.dt.float32
R32 = mybir.dt.float32r
BF16 = mybir.dt.bfloat16
EXPF = mybir.ActivationFunctionType.Exp
MULT = mybir.AluOpType.mult
ADD = mybir.AluOpType.add
BF = ml_dtypes.bfloat16

B, T, C, H, D = 4, 2048, 1024, 16, 64
HL = 8            # local heads per core
NP = HL // 2      # head pairs per core
KT = C // 128     # contraction tiles for projections
TT = T // 128     # 128-row tiles of T
QC = T // 512     # 512-col chunks of T
SCALE = 1.0 / math.sqrt(D)

_CACHE = {}
_DEBUG_DUMP = False


def _build_nc():
    nc = bacc.Bacc("TRN2", debug=False, num_devices=8)

    xT_d = nc.dram_tensor("xT", [KT, 128, T], R32, kind="ExternalInput").ap()
    wq_d = nc.dram_tensor("wq", [128, NP, KT, 128], R32, kind="ExternalInput").ap()
    wk_d = nc.dram_tensor("wk", [128, NP, KT, 128], R32, kind="ExternalInput").ap()
    wv_d = nc.dram_tensor("wv", [128, KT, 512], R32, kind="ExternalInput").ap()
    wo_d = nc.dram_tensor("wo", [128, NP, C], BF16, kind="ExternalInput").ap()
    cos_d = nc.dram_tensor("cosT", [128, T], F32, kind="ExternalInput").ap()
    sin_d = nc.dram_tensor("sinT", [128, T], F32, kind="ExternalInput").ap()
    psw_d = nc.dram_tensor("psw", [128, 128], BF16, kind="ExternalInput").ap()
    e64_d = nc.dram_tensor("e64", [128, 64], R32, kind="ExternalInput").ap()
    msk_d = nc.dram_tensor("msk", [128, 4, 512], BF16, kind="ExternalInput").ap()
    out_d = nc.dram_tensor("out", [T, C], F32, kind="ExternalOutput").ap()
    if _DEBUG_DUMP:
        dbg_qk = {
            (p, s): nc.dram_tensor(f"dbg_{s}{p}", [128, T], BF16,
                                   kind="ExternalOutput").ap()
            for p in range(NP) for s in "qk"
        }
        dbg_v = nc.dram_tensor("dbg_v", [128, TT, HL, D + 1], BF16,
                               kind="ExternalOutput").ap()
        dbg_y = {
            r: nc.dram_tensor(f"dbg_y{r}", [128, T], BF16,
                              kind="ExternalOutput").ap()
            for r in range(NP)
        }

    with tile.TileContext(nc) as tc:
        with ExitStack() as ctx:
            pers = ctx.enter_context(tc.tile_pool(name="pers", bufs=1))
            qkT = {}
            for p in range(NP):
                for s in "qk":
                    qkT[(p, s)] = pers.tile([128, T], BF16, name=f"qkT_{p}_{s}")
            yT = [pers.tile([128, T], BF16, name=f"yT_{r}") for r in range(NP)]
            vext = pers.tile([128, TT, HL, D + 1], BF16)
            wq_sb = pers.tile([128, NP, KT, 128], R32)
            wk_sb = pers.tile([128, NP, KT, 128], R32)
            wv_sb = pers.tile([128, KT, 512], R32)
            wo_sb = pers.tile([128, NP, C], BF16)
            cos_sb = pers.tile([128, T], F32)
            sin_sb = pers.tile([128, T], F32)
            psw_sb = pers.tile([128, 128], BF16)
            e64_sb = pers.tile([128, 64], R32)
            msk_sb = pers.tile([128, 4, 512], BF16)
            rrecA = pers.tile([128, 512], R32)
            rrecB = pers.tile([128, 512], R32)

            nc.sync.dma_start(wv_sb[:], wv_d)
            nc.sync.dma_start(wq_sb[:], wq_d)
            nc.sync.dma_start(wk_sb[:], wk_d)
            nc.sync.dma_start(wo_sb[:], wo_d)
            nc.sync.dma_start(cos_sb[:], cos_d)
            nc.sync.dma_start(sin_sb[:], sin_d)
            nc.sync.dma_start(psw_sb[:], psw_d)
            nc.sync.dma_start(e64_sb[:], e64_d)
            nc.sync.dma_start(msk_sb[:], msk_d)
            nc.gpsimd.memset(vext[:, :, :, D], 1.0)
            nc.gpsimd.memset(rrecA[:].bitcast(F32), 0.0)
            nc.gpsimd.memset(rrecB[:].bitcast(F32), 0.0)

            with (
                tc.tile_pool(name="xp", bufs=2) as xp,
                tc.tile_pool(name="sbp", bufs=2) as sbp,
                tc.tile_pool(name="mmp", bufs=2, space="PSUM") as mmp,
                tc.tile_pool(name="scp", bufs=2, space="PSUM") as scp,
                tc.tile_pool(name="pyp", bufs=1, space="PSUM") as pyp,
            ):
                xc_tiles = {}

                def emit_xdma(qc):
                    xc = xp.tile([128, KT, 512], R32, tag="xc", name=f"xc{qc}")
                    nc.sync.dma_start(
                        xc[:],
                        xT_d[:, :, qc * 512 : (qc + 1) * 512].rearrange(
                            "k c t -> c k t"
                        ),
                    )
                    xc_tiles[qc] = xc

                def emit_v2(qc, half):
                    xc = xc_tiles[qc]
                    for i in (2 * half, 2 * half + 1):
                        tt = 4 * qc + i
                        ps = mmp.tile([128, 512], F32, tag="mm", name=f"psv{tt}")
                        for kt in range(KT):
                            nc.tensor.matmul(
                                ps[:],
                                xc[:, kt, i * 128 : (i + 1) * 128],
                                wv_sb[:, kt],
                                start=(kt == 0),
                                stop=(kt == KT - 1),
                            )
                        nc.vector.tensor_copy(vext[:, tt, :, 0:D], ps[:])

                def emit_qk(qc, pair):
                    lo = qc * 512
                    xc = xc_tiles[qc]
                    pss = {}
                    for s, w_sb in (("q", wq_sb), ("k", wk_sb)):
                        ps = mmp.tile([128, 512], F32, tag="mm", name=f"ps{s}")
                        for kt in range(KT):
                            nc.tensor.matmul(
                                ps[:], w_sb[:, pair, kt], xc[:, kt],
                                start=(kt == 0), stop=(kt == KT - 1),
                            )
                        pss[s] = ps
                    prs = {}
                    for s in "qk":
                        ps = pss[s]
                        dst = qkT[(pair, s)][:, lo : lo + 512]
                        u = sbp.tile([128, 512], BF16, tag="u", name="u")
                        nc.vector.tensor_tensor(
                            u[:], ps[:], sin_sb[:, lo : lo + 512], MULT
                        )
                        nc.vector.tensor_tensor(
                            dst, ps[:], cos_sb[:, lo : lo + 512], MULT
                        )
                        pr = mmp.tile([128, 512], F32, tag="mm", name="pr")
                        nc.tensor.matmul(pr[:], psw_sb[:], u[:], start=True, stop=True)
                        prs[s] = pr
                    for s in "qk":
                        dst = qkT[(pair, s)][:, lo : lo + 512]
                        nc.vector.tensor_tensor(dst, prs[s][:], dst, ADD)

                def emit_att_kts(qc, p, psyA, psyB):
                    lo, hi = qc * 512, (qc + 1) * 512
                    nkt = (qc + 1) * 4
                    qTt = qkT[(p, "q")]
                    kTt = qkT[(p, "k")]

                    def emit_sc(kt):
                        klo, khi = kt * 128, (kt + 1) * 128
                        off = klo - lo
                        tr = off if off > 0 else 0
                        ps2 = scp.tile([128, 1024], F32, tag="sc", name="ps2")
                        p3 = ps2[:].rearrange("p (h n) -> p h n", h=2)
                        nc.tensor.matmul(
                            ps2[:, tr:512],
                            kTt[0:64, klo:khi], qTt[0:64, lo + tr : hi],
                            start=True, stop=True,
                        )
                        nc.tensor.matmul(
                            ps2[:, 512 + tr : 1024],
                            kTt[64:128, klo:khi], qTt[64:128, lo + tr : hi],
                            start=True, stop=True,
                        )
                        aAB = sbp.tile(
                            [128, 1024], BF16, tag="a", bufs=3, name="aAB"
                        )
                        a3 = aAB[:].rearrange("p (h n) -> p h n", h=2)
                        nc.scalar.activation(
                            a3[:, :, tr:512], p3[:, :, tr:512], EXPF, scale=SCALE
                        )
                        if off >= 0:
                            mi = off // 128
                            for h in range(2):
                                sl = a3[:, h, tr:512]
                                nc.vector.scalar_tensor_tensor(
                                    sl, sl, 1.0, msk_sb[:, mi, tr:512], MULT, MULT
                                )
                        return aAB, tr

                    def emit_av(kt, aAB, tr):
                        first, last = kt == 0, kt == nkt - 1
                        nc.tensor.matmul(
                            psyA[:, tr:512], vext[:, kt, 2 * p, :],
                            aAB[:, tr:512], start=first, stop=last,
                        )
                        nc.tensor.matmul(
                            psyB[:, tr:512], vext[:, kt, 2 * p + 1, :],
                            aAB[:, 512 + tr : 1024], start=first, stop=last,
                        )

                    prev = None
                    for kt in range(nkt):
                        cur = emit_sc(kt)
                        if prev is not None:
                            emit_av(kt - 1, *prev)
                        prev = cur
                    emit_av(nkt - 1, *prev)

                def emit_norm(qc, p, psyA, psyB):
                    lo, hi = qc * 512, (qc + 1) * 512
                    # denominator row -> SBUF (native single-partition copy),
                    # broadcast via the fp32r selector matmul, then approx-
                    # reciprocal on the 64-partition broadcast (the custom DVE
                    # op mis-addresses partition-offset outputs, so it only
                    # ever runs on base-0 tiles here)
                    nc.vector.tensor_copy(rrecA[64:65, :], psyA[64:65, :])
                    nc.vector.tensor_copy(rrecB[64:65, :], psyB[64:65, :])
                    pbcA = mmp.tile([128, 512], F32, tag="mm", name="pbcA")
                    nc.tensor.matmul(
                        pbcA[0:64, :], e64_sb[:], rrecA[:], start=True, stop=True
                    )
                    pbcB = mmp.tile([128, 512], F32, tag="mm", name="pbcB")
                    nc.tensor.matmul(
                        pbcB[0:64, :], e64_sb[:], rrecB[:], start=True, stop=True
                    )
                    bcA = sbp.tile([64, 512], F32, tag="bc", name="bcA")
                    nc.vector.tensor_copy(bcA[:], pbcA[0:64, :])
                    bcB = sbp.tile([64, 512], F32, tag="bc", name="bcB")
                    nc.vector.tensor_copy(bcB[:], pbcB[0:64, :])
                    brA = sbp.tile([64, 512], F32, tag="br", name="brA")
                    nc.vector.reciprocal_approx_fast(brA[:], bcA[:])
                    brB = sbp.tile([64, 512], F32, tag="br", name="brB")
                    nc.vector.reciprocal_approx_fast(brB[:], bcB[:])
                    nc.vector.tensor_tensor(
                        yT[p][0:64, lo:hi], psyA[0:64, :], brA[:], MULT
                    )
                    tb = sbp.tile([64, 512], BF16, tag="tb", name="tb")
                    nc.vector.tensor_tensor(tb[:], psyB[0:64, :], brB[:], MULT)
                    nc.sync.dma_start(yT[p][64:128, lo:hi], tb[:])

                def emit_out(qc):
                    for i in range(4):
                        mlo = (4 * qc + i) * 128
                        for cc in range(2):
                            clo = cc * 512
                            ps = mmp.tile([128, 512], F32, tag="mm", name="pso")
                            for r in range(NP):
                                nc.tensor.matmul(
                                    ps[:],
                                    yT[r][:, mlo : mlo + 128],
                                    wo_sb[:, r, clo : clo + 512],
                                    start=(r == 0), stop=(r == NP - 1),
                                )
                            ob = sbp.tile([128, 512], F32, tag="ob", name="ob")
                            nc.vector.tensor_copy(ob[:], ps[:])
                            nc.sync.dma_start(
                                out_d[mlo : mlo + 128, clo : clo + 512], ob[:]
                            )

                # prologue: chunk 0 projections
                emit_xdma(0)
                emit_v2(0, 0)
                emit_v2(0, 1)
                for pair in range(NP):
                    emit_qk(0, pair)

                for qc in range(QC):
                    nqc = qc + 1
                    has_next = nqc < QC
                    if has_next:
                        emit_xdma(nqc)
                    # per-pair: attention kts, then a projection chunk for the
                    # next qc (fills PE while DVE/Act work), then normalization
                    items = (
                        [
                            lambda: emit_v2(nqc, 0),
                            lambda: emit_v2(nqc, 1),
                            lambda: emit_qk(nqc, 0),
                            lambda: emit_qk(nqc, 1),
                        ]
                        if has_next
                        else [None] * 4
                    )
                    for p in range(NP):
                        psyA = pyp.tile([65, 512], F32, tag="pyA", name="psyA")
                        psyB = pyp.tile([65, 512], F32, tag="pyB", name="psyB")
                        emit_att_kts(qc, p, psyA, psyB)
                        if items[p] is not None:
                            items[p]()
                        emit_norm(qc, p, psyA, psyB)
                    if has_next:
                        emit_qk(nqc, 2)
                        emit_qk(nqc, 3)
                    emit_out(qc)

                if _DEBUG_DUMP:
                    for p in range(NP):
                        for s in "qk":
                            nc.sync.dma_start(dbg_qk[(p, s)], qkT[(p, s)][:])
                        nc.sync.dma_start(dbg_y[p], yT[p][:])
                    nc.sync.dma_start(dbg_v, vext[:])

    nc.compile()
    return nc


def _host_tables():
    half = D // 2
    freq = np.exp(-math.log(10000.0) * np.arange(half) / half).astype(np.float64)
    ang = np.arange(T, dtype=np.float64)[None, :] * freq[:, None]  # [32, T]
    cos32 = np.cos(ang).astype(np.float32)
    sin32 = np.sin(ang).astype(np.float32)
    cosT = np.tile(cos32, (4, 1))                                   # [128, T]
    sinT = np.concatenate([sin32, -sin32, sin32, -sin32], axis=0)   # [128, T]
    psw = np.zeros((128, 128), np.float32)
    psw[np.arange(128) ^ 32, np.arange(128)] = 1.0
    e64 = np.zeros((128, 64), np.float32)
    e64[64, :] = 1.0
    kk = np.arange(128)[:, None, None]
    ii = np.arange(4)[None, :, None]
    qq = np.arange(512)[None, None, :]
    msk = (qq >= kk + ii * 128).astype(np.float32)
    return cosT, sinT, psw.astype(BF), e64, msk.astype(BF)


def _pack_weights(w_qkv, w_out, hg):
    lo, hi = hg * HL, (hg + 1) * HL
    wqf = w_qkv[:, 0:C].reshape(C, H, D)[:, lo:hi]       # [C, 8, D]
    wkf = w_qkv[:, C : 2 * C].reshape(C, H, D)[:, lo:hi]
    wvf = w_qkv[:, 2 * C : 3 * C].reshape(C, H, D)[:, lo:hi]

    def pack_qk(w):
        a = w.reshape(KT, 128, NP, 2, D)
        return np.ascontiguousarray(
            a.transpose(1, 2, 0, 3, 4).reshape(128, NP, KT, 128)
        )

    wq = pack_qk(wqf)
    wk = pack_qk(wkf)
    wv = np.ascontiguousarray(
        wvf.reshape(KT, 128, HL * D).transpose(1, 0, 2)
    )
    wo_l = w_out.reshape(H, D, C)[lo:hi].reshape(NP, 128, C)
    wo = np.ascontiguousarray(wo_l.transpose(1, 0, 2)).astype(BF)
    return wq, wk, wv, wo


def _in_maps(x, w_qkv, w_out):
    x = np.asarray(x, dtype=np.float32)
    w_qkv = np.asarray(w_qkv, dtype=np.float32)
    w_out = np.asarray(w_out, dtype=np.float32)
    cosT, sinT, psw, e64, msk = _host_tables()
    packs = [_pack_weights(w_qkv, w_out, hg) for hg in range(2)]
    xTs = [
        np.ascontiguousarray(x[b].T).reshape(KT, 128, T) for b in range(B)
    ]
    in_maps = []
    for c in range(8):
        b, hg = c // 2, c % 2
        wq, wk, wv, wo = packs[hg]
        in_maps.append(
            {
                "xT": xTs[b], "wq": wq, "wk": wk, "wv": wv, "wo": wo,
                "cosT": cosT, "sinT": sinT, "psw": psw, "e64": e64,
                "msk": msk,
            }
        )
    return in_maps


def kernel(x, w_qkv, w_out):
    if "nc" not in _CACHE:
        _CACHE["nc"] = _build_nc()
    nc = _CACHE["nc"]

    in_maps = _in_maps(x, w_qkv, w_out)
    res = run_bass_kernel_spmd(nc, in_maps, core_ids=list(range(8)))
    outs = [np.asarray(res.results[c]["out"], np.float32) for c in range(8)]
    y = np.stack([outs[2 * b] + outs[2 * b + 1] for b in range(B)], axis=0)
    return y.astype(np.float32)
